# revision 29
# baseline (speedup 1.0000x reference)
"""GAT layer (nn_GATLayer) Trainium2 Bass kernel, 8-core SPMD — v4.

Math (exp(s_src) cancels in the softmax):
  out[n,h,:] = ELU( (sum_m A[n,m] e[m,h] ft[m,h,:]) / (sum_m A[n,m] e[m,h]) )
  e[m,h] = exp(s_dst[m,h] + b[h]),  ft = features @ W_lin.T + b_lin
Per core: Y = A_slab @ G with G = [e*ft | e]  ([8192, 130]).

v4 over v3:
- adjacency slab shipped as fp8e4m3 ({0,1,2} exact): halves HBM traffic
  (8.4MB/core) and LDWEIGHTS time (FWL loads 4 fp8/cycle).
- preproc writes G straight from PSUM (exp on ACT, dual-head multiply on
  DVE, per 3-m-tile group) — no pp buffer, no ACT copy drains.
- main loop is mt-outer with all 8 destination-row accumulators packed
  into 3 PSUM banks (130 fp32 at offsets 0/130/260; first matmul in a
  bank start=True, later chains start=False relying on per-element
  has_written overwrite semantics), so a single mt sweep feeds all 8
  n-tiles and the adjacency streams through the PE exactly once.
- all 8 adjacency chunks prefetched up front (contiguous 1MB DMAs).
"""

import numpy as np
import ml_dtypes

import concourse.bass as bass
import concourse.mybir as mybir
import concourse.tile as tile
from concourse import bacc
from concourse.bass_utils import run_bass_kernel_spmd

F32 = mybir.dt.float32
BF16 = mybir.dt.bfloat16
FP8 = mybir.dt.float8e4
AF = mybir.ActivationFunctionType

N = 8192
IN_DIM = 64
OUT_DIM = 64
HEADS = 2
NCORES = 8
ROWS = N // NCORES           # 1024 destination rows per core
NT = ROWS // 128             # 8 n-tiles per core
MT = N // 128                # 64 m-tiles
C = HEADS * OUT_DIM + HEADS  # 130 columns of G
FT65 = IN_DIM + 1
NP_FP8 = np.dtype(ml_dtypes.float8_e4m3)
GRP = 3                      # m-tiles per PSUM bank (3*130*4B<=2KB)
PBANKS = 2                   # preproc PSUM banks per group
PGRP = GRP * PBANKS          # preproc m-tiles per group (12)
NGRP = (MT + PGRP - 1) // PGRP
BANKE = 512                  # fp32 elems per PSUM bank
MCHUNK = 8                   # m-tiles per adjacency DMA (1MB each)
TIME_UNROLL = 4              # bodies per For_i iteration in timing builds


def _ap(t, off, dims):
    return bass.AP(tensor=t.tensor, offset=t.offset + off,
                   ap=[list(t.ap[0])] + dims)


def build_program(reps: int = 1, timing: bool = False, variant: str = "full",
                  mchunk: int = MCHUNK, rings: int = 2):
    """timing=True: adjt becomes an Internal DRAM tensor (not transferred
    over the axon tunnel; HW timing is value-independent) and the reps
    run in a tc.For_i hardware loop so the program stays small while the
    kernel body executes `reps` times on device.

    variant (timing only): 'full' = whole body; 'mm_dma' = adjacency DMA
    + main matmuls only; 'mm' = main matmuls only (garbage at/g)."""
    nc = bacc.Bacc("TRN2", target_bir_lowering=False, debug=False,
                   num_devices=NCORES)

    # adjt host layout: [128 (m within tile), MT, NT, 128 (n within tile)]
    adjt = nc.dram_tensor("adjt", [128, MT * NT * 128], FP8,
                          kind="Internal" if timing else "ExternalInput").ap()
    ftcat = nc.dram_tensor("ftcat", [FT65, N], BF16,
                           kind="ExternalInput").ap()
    wcat = nc.dram_tensor("wcat", [FT65, C], BF16, kind="ExternalInput").ap()
    out = nc.dram_tensor("out", [ROWS, HEADS * OUT_DIM], F32,
                         kind="ExternalOutput").ap()
    # out rows n = t*128 + p as [p, t, c] for the single batched store
    out_pt = bass.AP(tensor=out.tensor, offset=0,
                     ap=[[HEADS * OUT_DIM, 128],
                         [128 * HEADS * OUT_DIM, NT],
                         [1, HEADS * OUT_DIM]])

    with tile.TileContext(nc) as tc:
        with tc.tile_pool(name="const", bufs=1) as const, \
             tc.tile_pool(name="gpool", bufs=2) as gpool, \
             tc.tile_pool(name="atp", bufs=2) as atp, \
             tc.tile_pool(name="ep", bufs=2) as ep, \
             tc.tile_pool(name="ps_g", bufs=2, space="PSUM") as ps_g, \
             tc.tile_pool(name="ps_y", bufs=1, space="PSUM") as ps_y_p:

            ft_sb = const.tile([FT65, N], BF16)
            nc.sync.dma_start(out=ft_sb, in_=ftcat)
            wc_sb = const.tile([FT65, C], BF16)
            nc.sync.dma_start(out=wc_sb, in_=wcat)
            if variant != "full":
                at_c = const.tile([128, MT, NT, 128], FP8, name="at_c")
                atc_f = at_c.rearrange("p a b c -> p (a b c)")
                half = MT * NT * 128 // 2
                nc.vector.memset(atc_f[:, :half], 0)
                nc.vector.memset(atc_f[:, half:], 0)
                g_c = const.tile([128, MT, C], BF16, name="g_c")
                nc.vector.memset(g_c.rearrange("p a b -> p (a b)"), 0)

            def body(_i=None):
                # ---- adjacency prefetch: 8 contiguous 1MB DMAs
                if variant == "full":
                    at = atp.tile([128, MT, NT, 128], FP8, name="at")
                else:
                    at = at_c
                atf = at.rearrange("p a b c -> p (a b c)")
                if variant != "mm":
                    engs = [nc.sync, nc.scalar, nc.gpsimd][:rings]
                    for ck in range(MT // mchunk):
                        w = mchunk * NT * 128
                        eng = engs[ck % rings]
                        eng.dma_start(out=atf[:, ck * w:(ck + 1) * w],
                                      in_=adjt[:, ck * w:(ck + 1) * w])

                # ---- preproc: G = [e*ft | e] straight from PSUM, per
                # 3-m-tile group; main-loop matmuls for group k-1 are
                # emitted between group k's ops so the PE alternates
                # (preproc grp k || DVE muls grp k || main MMs grp k-1)
                # instead of serializing the two phases.
                g = gpool.tile([128, MT, C], BF16, name="g") \
                    if variant == "full" else g_c
                ps_y = ps_y_p.tile([128, 3, BANKE], F32, name="ps_y")

                def main_mms(m0, nm):
                    for mt in range(m0, m0 + nm):
                        for t in range(NT):
                            b, j = divmod(t, GRP)
                            nc.tensor.matmul(
                                _ap(ps_y, b * BANKE + j * C, [[1, C]]),
                                at[:, mt, t, :], g[:, mt, :],
                                start=(mt == 0 and j == 0),
                                stop=(mt == MT - 1),
                                skip_group_check=True)

                def preproc_grp(m0, nm):
                    # MMs into a 4-bank psg (3 m-tiles per bank), then one
                    # batched exp + two batched muls over the whole group:
                    # fewer DVE/ACT instructions -> fewer inter-instruction
                    # SBUF bubbles (the trn2 errata charges one per op).
                    psg = ps_g.tile([128, PBANKS, BANKE], F32, name="psg",
                                    tag="psg")
                    for j in range(nm):
                        b, s = divmod(j, GRP)
                        nc.tensor.matmul(
                            _ap(psg, b * BANKE + s * C, [[1, C]]),
                            ft_sb[:, (m0 + j) * 128:(m0 + j + 1) * 128],
                            wc_sb, start=True, stop=True)
                    spans = [(0, nm)] if nm == PGRP else \
                        [(b, min(GRP, nm - b * GRP))
                         for b in range((nm + GRP - 1) // GRP)]
                    for b0, cnt in spans:
                        if cnt == nm:  # full group: 3-level AP over banks
                            pdims = [[BANKE, PBANKS], [C, GRP]]
                            gdims = [[GRP * C, PBANKS], [C, GRP]]
                        else:          # remainder: per-bank ops
                            pdims = [[BANKE, 1], [C, cnt]]
                            gdims = [[GRP * C, 1], [C, cnt]]
                        po = b0 * BANKE
                        go = (m0 + b0 * GRP) * C
                        nc.scalar.activation(
                            _ap(g, go + 128, gdims + [[1, HEADS]]),
                            _ap(psg, po + 128, pdims + [[1, HEADS]]),
                            AF.Exp)
                        for h in range(HEADS):
                            nc.vector.tensor_mul(
                                _ap(g, go + h * 64, gdims + [[1, 64]]),
                                _ap(psg, po + h * 64, pdims + [[1, 64]]),
                                _ap(g, go + 128 + h, gdims + [[0, 64]]))

                if variant == "full":
                    for k in range(NGRP):
                        m0 = k * PGRP
                        preproc_grp(m0, min(PGRP, MT - m0))
                        if k > 0:
                            main_mms((k - 1) * PGRP, PGRP)
                    main_mms((NGRP - 1) * PGRP, MT - (NGRP - 1) * PGRP)
                else:
                    main_mms(0, MT)

                if variant != "full":
                    return
                # ---- epilogue: one batched reciprocal (9th slot is
                # garbage, unused), per-bank normalize muls, ELU, store
                obuf = ep.tile([128, NT, 128], F32, name="obuf")
                r2 = ep.tile([128, 3 * GRP, HEADS], F32, name="r2")
                nc.vector.reciprocal(
                    _ap(r2, 0, [[GRP * HEADS, 2], [HEADS, GRP], [1, HEADS]]),
                    _ap(ps_y, 128, [[BANKE, 2], [C, GRP], [1, HEADS]]))
                nc.vector.reciprocal(
                    _ap(r2, 2 * GRP * HEADS, [[HEADS, 2], [1, HEADS]]),
                    _ap(ps_y, 2 * BANKE + 128, [[C, 2], [1, HEADS]]))
                for b in range(3):
                    nt = min(GRP, NT - b * GRP)
                    nc.vector.tensor_mul(
                        _ap(obuf, b * GRP * 128,
                            [[128, nt], [64, HEADS], [1, 64]]),
                        _ap(ps_y, b * BANKE, [[C, nt], [64, HEADS], [1, 64]]),
                        _ap(r2, b * GRP * HEADS,
                            [[HEADS, nt], [1, HEADS], [0, 64]]))

                # ELU(x) = max(x, exp(min(x,0)) - 1)   (e^x >= 1+x)
                of = obuf.rearrange("p a b -> p (a b)")
                mn = ep.tile([128, NT * 128], BF16, name="mn")
                nc.vector.tensor_scalar_min(mn, of, 0.0)
                ex = ep.tile([128, NT * 128], F32, name="ex")
                nc.scalar.activation(ex, mn, AF.Exp)
                nc.vector.scalar_tensor_tensor(
                    of, ex, -1.0, of,
                    mybir.AluOpType.add, mybir.AluOpType.max)
                nc.sync.dma_start(out=out_pt, in_=obuf)

            if timing:
                assert reps % TIME_UNROLL == 0
                with tc.For_i(0, reps // TIME_UNROLL):
                    for _u in range(TIME_UNROLL):
                        body()
            else:
                for _rep in range(reps):
                    body()

    nc.compile()
    return nc


def make_in_maps(adj, features, W_attn, b_attn, W_lin, b_lin):
    adj = np.asarray(adj, dtype=np.float32)
    features = np.asarray(features, dtype=np.float32)
    W_attn = np.asarray(W_attn, dtype=np.float32)
    b_attn = np.asarray(b_attn, dtype=np.float32)
    W_lin = np.asarray(W_lin, dtype=np.float32)
    b_lin = np.asarray(b_lin, dtype=np.float32)

    BF = np.dtype(ml_dtypes.bfloat16)
    ftcat = np.concatenate([features.T,
                            np.ones((1, N), np.float32)], axis=0)
    ftcat = np.ascontiguousarray(ftcat).astype(BF)
    wcat = np.zeros((FT65, C), np.float32)
    wcat[:IN_DIM, 0:HEADS * OUT_DIM] = W_lin.T
    wcat[:IN_DIM, HEADS * OUT_DIM:] = W_attn[:, IN_DIM:].T
    wcat[IN_DIM, 0:HEADS * OUT_DIM] = b_lin
    wcat[IN_DIM, HEADS * OUT_DIM:] = b_attn
    wcat = wcat.astype(BF)

    A = adj.astype(NP_FP8)
    idx = np.arange(N)
    A[idx, idx] = (adj[idx, idx] + 1.0).astype(NP_FP8)

    in_maps = []
    for c in range(NCORES):
        slab = A[c * ROWS:(c + 1) * ROWS, :]       # [1024 n, 8192 m]
        # -> [128 (m in tile), MT, NT, 128 (n in tile)]
        adjt = slab.reshape(NT, 128, MT, 128).transpose(3, 2, 0, 1)
        adjt = np.ascontiguousarray(adjt).reshape(128, MT * NT * 128)
        in_maps.append({"adjt": adjt, "ftcat": ftcat, "wcat": wcat})
    return in_maps


_CACHED = {}


def _get_program(reps=1, timing=False, variant="full", mchunk=MCHUNK,
                 rings=2):
    key = (reps, timing, variant, mchunk, rings)
    if key not in _CACHED:
        _CACHED[key] = build_program(reps, timing=timing, variant=variant,
                                     mchunk=mchunk, rings=rings)
    return _CACHED[key]


def run_on_device(in_maps, reps=1, timing=False, variant="full",
                  mchunk=MCHUNK, rings=2, **kw):
    nc = _get_program(reps, timing=timing, variant=variant, mchunk=mchunk,
                      rings=rings)
    if timing:
        in_maps = [{k: v for k, v in m.items() if k != "adjt"}
                   for m in in_maps]
    res = run_bass_kernel_spmd(nc, in_maps, core_ids=list(range(NCORES)), **kw)
    return res


def kernel(adj, features, W_attn, b_attn, W_lin, b_lin):
    in_maps = make_in_maps(adj, features, W_attn, b_attn, W_lin, b_lin)
    res = run_on_device(in_maps, reps=1)
    return np.concatenate([res.results[c]["out"] for c in range(NCORES)],
                          axis=0)


# revision 33
# speedup vs baseline: 1.1582x; 1.1582x over previous
"""GAT layer (nn_GATLayer) Trainium2 Bass kernel, 8-core SPMD — v4.

Math (exp(s_src) cancels in the softmax):
  out[n,h,:] = ELU( (sum_m A[n,m] e[m,h] ft[m,h,:]) / (sum_m A[n,m] e[m,h]) )
  e[m,h] = exp(s_dst[m,h] + b[h]),  ft = features @ W_lin.T + b_lin
Per core: Y = A_slab @ G with G = [e*ft | e]  ([8192, 130]).

v4 over v3:
- adjacency slab shipped as fp8e4m3 ({0,1,2} exact): halves HBM traffic
  (8.4MB/core) and LDWEIGHTS time (FWL loads 4 fp8/cycle).
- preproc writes G straight from PSUM (exp on ACT, dual-head multiply on
  DVE, per 3-m-tile group) — no pp buffer, no ACT copy drains.
- main loop is mt-outer with all 8 destination-row accumulators packed
  into 3 PSUM banks (130 fp32 at offsets 0/130/260; first matmul in a
  bank start=True, later chains start=False relying on per-element
  has_written overwrite semantics), so a single mt sweep feeds all 8
  n-tiles and the adjacency streams through the PE exactly once.
- all 8 adjacency chunks prefetched up front (contiguous 1MB DMAs).
"""

import numpy as np
import ml_dtypes

import concourse.bass as bass
import concourse.mybir as mybir
import concourse.tile as tile
from concourse import bacc
from concourse.bass_utils import run_bass_kernel_spmd

F32 = mybir.dt.float32
BF16 = mybir.dt.bfloat16
FP8 = mybir.dt.float8e4
AF = mybir.ActivationFunctionType

N = 8192
IN_DIM = 64
OUT_DIM = 64
HEADS = 2
NCORES = 8
ROWS = N // NCORES           # 1024 destination rows per core
NT = ROWS // 128             # 8 n-tiles per core
MT = N // 128                # 64 m-tiles
C = HEADS * OUT_DIM + HEADS  # 130 columns of G
FT65 = IN_DIM + 1
NP_FP8 = np.dtype(ml_dtypes.float8_e4m3)
GRP = 3                      # m-tiles per PSUM bank (3*130*4B<=2KB)
NGRP = (MT + GRP - 1) // GRP
MCHUNK = 8                   # m-tiles per adjacency DMA (1MB each)
TIME_UNROLL = 4              # bodies per For_i iteration in timing builds


def _ap(t, off, dims):
    return bass.AP(tensor=t.tensor, offset=t.offset + off,
                   ap=[list(t.ap[0])] + dims)


def build_program(reps: int = 1, timing: bool = False, variant: str = "full",
                  mchunk: int = MCHUNK, rings: int = 2):
    """timing=True: adjt becomes an Internal DRAM tensor (not transferred
    over the axon tunnel; HW timing is value-independent) and the reps
    run in a tc.For_i hardware loop so the program stays small while the
    kernel body executes `reps` times on device.

    variant (timing only): 'full' = whole body; 'mm_dma' = adjacency DMA
    + main matmuls only; 'mm' = main matmuls only (garbage at/g)."""
    nc = bacc.Bacc("TRN2", target_bir_lowering=False, debug=False,
                   num_devices=NCORES)

    # adjt host layout: [128 (m within tile), MT, NT, 128 (n within tile)]
    adjt = nc.dram_tensor("adjt", [128, MT * NT * 128], FP8,
                          kind="Internal" if timing else "ExternalInput").ap()
    ftcat = nc.dram_tensor("ftcat", [FT65, N], BF16,
                           kind="ExternalInput").ap()
    wcat = nc.dram_tensor("wcat", [FT65, C], BF16, kind="ExternalInput").ap()
    out = nc.dram_tensor("out", [ROWS, HEADS * OUT_DIM], F32,
                         kind="ExternalOutput").ap()
    # out rows n = t*128 + p as [p, t, c] for the single batched store
    out_pt = bass.AP(tensor=out.tensor, offset=0,
                     ap=[[HEADS * OUT_DIM, 128],
                         [128 * HEADS * OUT_DIM, NT],
                         [1, HEADS * OUT_DIM]])

    with tile.TileContext(nc) as tc:
        with tc.tile_pool(name="const", bufs=1) as const, \
             tc.tile_pool(name="gpool", bufs=2) as gpool, \
             tc.tile_pool(name="atp", bufs=2) as atp, \
             tc.tile_pool(name="ep", bufs=2) as ep, \
             tc.tile_pool(name="ps_g", bufs=4, space="PSUM") as ps_g, \
             tc.tile_pool(name="ps_y", bufs=1, space="PSUM") as ps_y_p:

            ft_sb = const.tile([FT65, N], BF16)
            nc.sync.dma_start(out=ft_sb, in_=ftcat)
            wc_sb = const.tile([FT65, C], BF16)
            nc.sync.dma_start(out=wc_sb, in_=wcat)
            if variant != "full":
                at_c = const.tile([128, MT, NT, 128], FP8, name="at_c")
                atc_f = at_c.rearrange("p a b c -> p (a b c)")
                half = MT * NT * 128 // 2
                nc.vector.memset(atc_f[:, :half], 0)
                nc.vector.memset(atc_f[:, half:], 0)
                g_c = const.tile([128, MT, C], BF16, name="g_c")
                nc.vector.memset(g_c.rearrange("p a b -> p (a b)"), 0)

            def body(_i=None):
                # ---- adjacency prefetch: 8 contiguous 1MB DMAs
                if variant == "full":
                    at = atp.tile([128, MT, NT, 128], FP8, name="at")
                else:
                    at = at_c
                atf = at.rearrange("p a b c -> p (a b c)")
                if variant != "mm":
                    engs = [nc.sync, nc.scalar, nc.gpsimd][:rings]
                    for ck in range(MT // mchunk):
                        w = mchunk * NT * 128
                        eng = engs[ck % rings]
                        eng.dma_start(out=atf[:, ck * w:(ck + 1) * w],
                                      in_=adjt[:, ck * w:(ck + 1) * w])

                # ---- preproc: G = [e*ft | e] straight from PSUM,
                # per 3-m-tile group
                g = gpool.tile([128, MT, C], BF16, name="g") \
                    if variant == "full" else g_c
                for k in range(NGRP if variant == "full" else 0):
                    m0 = k * GRP
                    nm = min(GRP, MT - m0)
                    psg = ps_g.tile([128, GRP, C], F32, name="psg", tag="psg")
                    for j in range(nm):
                        nc.tensor.matmul(
                            psg[:, j, :],
                            ft_sb[:, (m0 + j) * 128:(m0 + j + 1) * 128],
                            wc_sb, start=True, stop=True)
                    # e-cols: exp(PSUM) -> g[:, m0:m0+nm, 128:130] (bf16)
                    nc.scalar.activation(
                        _ap(g, m0 * C + 128, [[C, nm], [1, HEADS]]),
                        _ap(psg, 128, [[C, nm], [1, HEADS]]),
                        AF.Exp)
                    # ft-cols: psg * e (broadcast bf16 e-cols; their
                    # rounding cancels in the softmax ratio)
                    for h in range(HEADS):
                        nc.vector.tensor_mul(
                            _ap(g, m0 * C + h * 64, [[C, nm], [1, 64]]),
                            _ap(psg, h * 64, [[C, nm], [1, 64]]),
                            _ap(g, m0 * C + 128 + h, [[C, nm], [0, 64]]))

                # ---- main: single mt sweep, 8 accumulators in 3 banks
                ps_y = [ps_y_p.tile([128, GRP, C], F32, name=f"psy{b}",
                                    tag=f"psy{b}") for b in range(3)]
                for mt in range(MT):
                    for t in range(NT):
                        b, j = divmod(t, GRP)
                        nc.tensor.matmul(
                            ps_y[b][:, j, :], at[:, mt, t, :], g[:, mt, :],
                            start=(mt == 0 and j == 0),
                            stop=(mt == MT - 1),
                            skip_group_check=True)

                if variant != "full":
                    return
                # ---- epilogue: normalize (batched per bank), ELU, store
                obuf = ep.tile([128, NT, 128], F32, name="obuf")
                r2 = ep.tile([128, NT, HEADS], F32, name="r2")
                for b in range(3):
                    nt = min(GRP, NT - b * GRP)
                    nc.vector.reciprocal(
                        _ap(r2, b * GRP * HEADS, [[HEADS, nt], [1, HEADS]]),
                        _ap(ps_y[b], 128, [[C, nt], [1, HEADS]]))
                    nc.vector.tensor_mul(
                        _ap(obuf, b * GRP * 128,
                            [[128, nt], [64, HEADS], [1, 64]]),
                        _ap(ps_y[b], 0, [[C, nt], [64, HEADS], [1, 64]]),
                        _ap(r2, b * GRP * HEADS,
                            [[HEADS, nt], [1, HEADS], [0, 64]]))

                # ELU(x) = max(x, exp(min(x,0)) - 1)   (e^x >= 1+x)
                of = obuf.rearrange("p a b -> p (a b)")
                mn = ep.tile([128, NT * 128], BF16, name="mn")
                nc.vector.tensor_scalar_min(mn, of, 0.0)
                ex = ep.tile([128, NT * 128], F32, name="ex")
                nc.scalar.activation(ex, mn, AF.Exp)
                nc.vector.scalar_tensor_tensor(
                    of, ex, -1.0, of,
                    mybir.AluOpType.add, mybir.AluOpType.max)
                nc.sync.dma_start(out=out_pt, in_=obuf)

            if timing:
                assert reps % TIME_UNROLL == 0
                with tc.For_i(0, reps // TIME_UNROLL):
                    for _u in range(TIME_UNROLL):
                        body()
            else:
                for _rep in range(reps):
                    body()

    nc.compile()
    return nc


def make_in_maps(adj, features, W_attn, b_attn, W_lin, b_lin):
    adj = np.asarray(adj, dtype=np.float32)
    features = np.asarray(features, dtype=np.float32)
    W_attn = np.asarray(W_attn, dtype=np.float32)
    b_attn = np.asarray(b_attn, dtype=np.float32)
    W_lin = np.asarray(W_lin, dtype=np.float32)
    b_lin = np.asarray(b_lin, dtype=np.float32)

    BF = np.dtype(ml_dtypes.bfloat16)
    ftcat = np.concatenate([features.T,
                            np.ones((1, N), np.float32)], axis=0)
    ftcat = np.ascontiguousarray(ftcat).astype(BF)
    wcat = np.zeros((FT65, C), np.float32)
    wcat[:IN_DIM, 0:HEADS * OUT_DIM] = W_lin.T
    wcat[:IN_DIM, HEADS * OUT_DIM:] = W_attn[:, IN_DIM:].T
    wcat[IN_DIM, 0:HEADS * OUT_DIM] = b_lin
    wcat[IN_DIM, HEADS * OUT_DIM:] = b_attn
    wcat = wcat.astype(BF)

    A = adj.astype(NP_FP8)
    idx = np.arange(N)
    A[idx, idx] = (adj[idx, idx] + 1.0).astype(NP_FP8)

    in_maps = []
    for c in range(NCORES):
        slab = A[c * ROWS:(c + 1) * ROWS, :]       # [1024 n, 8192 m]
        # -> [128 (m in tile), MT, NT, 128 (n in tile)]
        adjt = slab.reshape(NT, 128, MT, 128).transpose(3, 2, 0, 1)
        adjt = np.ascontiguousarray(adjt).reshape(128, MT * NT * 128)
        in_maps.append({"adjt": adjt, "ftcat": ftcat, "wcat": wcat})
    return in_maps


_CACHED = {}


def _get_program(reps=1, timing=False, variant="full", mchunk=MCHUNK,
                 rings=2):
    key = (reps, timing, variant, mchunk, rings)
    if key not in _CACHED:
        _CACHED[key] = build_program(reps, timing=timing, variant=variant,
                                     mchunk=mchunk, rings=rings)
    return _CACHED[key]


def run_on_device(in_maps, reps=1, timing=False, variant="full",
                  mchunk=MCHUNK, rings=2, **kw):
    nc = _get_program(reps, timing=timing, variant=variant, mchunk=mchunk,
                      rings=rings)
    if timing:
        in_maps = [{k: v for k, v in m.items() if k != "adjt"}
                   for m in in_maps]
    res = run_bass_kernel_spmd(nc, in_maps, core_ids=list(range(NCORES)), **kw)
    return res


def kernel(adj, features, W_attn, b_attn, W_lin, b_lin):
    in_maps = make_in_maps(adj, features, W_attn, b_attn, W_lin, b_lin)
    res = run_on_device(in_maps, reps=1)
    return np.concatenate([res.results[c]["out"] for c in range(NCORES)],
                          axis=0)


# revision 34
# speedup vs baseline: 1.1914x; 1.0286x over previous
"""GAT layer (nn_GATLayer) Trainium2 Bass kernel, 8-core SPMD — v4.

Math (exp(s_src) cancels in the softmax):
  out[n,h,:] = ELU( (sum_m A[n,m] e[m,h] ft[m,h,:]) / (sum_m A[n,m] e[m,h]) )
  e[m,h] = exp(s_dst[m,h] + b[h]),  ft = features @ W_lin.T + b_lin
Per core: Y = A_slab @ G with G = [e*ft | e]  ([8192, 130]).

v4 over v3:
- adjacency slab shipped as fp8e4m3 ({0,1,2} exact): halves HBM traffic
  (8.4MB/core) and LDWEIGHTS time (FWL loads 4 fp8/cycle).
- preproc writes G straight from PSUM (exp on ACT, dual-head multiply on
  DVE, per 3-m-tile group) — no pp buffer, no ACT copy drains.
- main loop is mt-outer with all 8 destination-row accumulators packed
  into 3 PSUM banks (130 fp32 at offsets 0/130/260; first matmul in a
  bank start=True, later chains start=False relying on per-element
  has_written overwrite semantics), so a single mt sweep feeds all 8
  n-tiles and the adjacency streams through the PE exactly once.
- all 8 adjacency chunks prefetched up front as contiguous 1MB DMAs,
  alternating between the two HWDGE rings (nc.sync / nc.scalar): one
  ring caps at ~180 GB/s, two reach ~330-350 GB/s.
- ELU in 3 ops: max(x, exp(min(x,0)) - 1).

Measured (For_i timing build, reps 16 vs 4096 slope): ~41-45 us/rep vs
~100 us for the staged v3 baseline. Pure-matmul floor 16.3 us,
matmul+adjacency-DMA floor ~30 us (memory-bound, as per target_regime).
"""

import numpy as np
import ml_dtypes

import concourse.bass as bass
import concourse.mybir as mybir
import concourse.tile as tile
from concourse import bacc
from concourse.bass_utils import run_bass_kernel_spmd

F32 = mybir.dt.float32
BF16 = mybir.dt.bfloat16
FP8 = mybir.dt.float8e4
AF = mybir.ActivationFunctionType

N = 8192
IN_DIM = 64
OUT_DIM = 64
HEADS = 2
NCORES = 8
ROWS = N // NCORES           # 1024 destination rows per core
NT = ROWS // 128             # 8 n-tiles per core
MT = N // 128                # 64 m-tiles
C = HEADS * OUT_DIM + HEADS  # 130 columns of G
FT65 = IN_DIM + 1
NP_FP8 = np.dtype(ml_dtypes.float8_e4m3)
GRP = 3                      # m-tiles per PSUM bank (3*130*4B<=2KB)
NGRP = (MT + GRP - 1) // GRP
MCHUNK = 8                   # m-tiles per adjacency DMA (1MB each)
TIME_UNROLL = 4              # bodies per For_i iteration in timing builds


def _ap(t, off, dims):
    return bass.AP(tensor=t.tensor, offset=t.offset + off,
                   ap=[list(t.ap[0])] + dims)


def build_program(reps: int = 1, timing: bool = False, variant: str = "full",
                  mchunk: int = MCHUNK, rings: int = 2):
    """timing=True: adjt becomes an Internal DRAM tensor (not transferred
    over the axon tunnel; HW timing is value-independent) and the reps
    run in a tc.For_i hardware loop so the program stays small while the
    kernel body executes `reps` times on device.

    variant (timing only): 'full' = whole body; 'mm_dma' = adjacency DMA
    + main matmuls only; 'mm' = main matmuls only (garbage at/g)."""
    nc = bacc.Bacc("TRN2", target_bir_lowering=False, debug=False,
                   num_devices=NCORES)

    # adjt host layout: [128 (m within tile), MT, NT, 128 (n within tile)]
    adjt = nc.dram_tensor("adjt", [128, MT * NT * 128], FP8,
                          kind="Internal" if timing else "ExternalInput").ap()
    ftcat = nc.dram_tensor("ftcat", [FT65, N], BF16,
                           kind="ExternalInput").ap()
    wcat = nc.dram_tensor("wcat", [FT65, C], BF16, kind="ExternalInput").ap()
    out = nc.dram_tensor("out", [ROWS, HEADS * OUT_DIM], F32,
                         kind="ExternalOutput").ap()
    # out rows n = t*128 + p as [p, t, c] for the single batched store
    out_pt = bass.AP(tensor=out.tensor, offset=0,
                     ap=[[HEADS * OUT_DIM, 128],
                         [128 * HEADS * OUT_DIM, NT],
                         [1, HEADS * OUT_DIM]])

    with tile.TileContext(nc) as tc:
        with tc.tile_pool(name="const", bufs=1) as const, \
             tc.tile_pool(name="gpool", bufs=2) as gpool, \
             tc.tile_pool(name="atp", bufs=2) as atp, \
             tc.tile_pool(name="ep", bufs=2) as ep, \
             tc.tile_pool(name="ps_g", bufs=4, space="PSUM") as ps_g, \
             tc.tile_pool(name="ps_y", bufs=1, space="PSUM") as ps_y_p:

            ft_sb = const.tile([FT65, N], BF16)
            nc.sync.dma_start(out=ft_sb, in_=ftcat)
            wc_sb = const.tile([FT65, C], BF16)
            nc.sync.dma_start(out=wc_sb, in_=wcat)
            if variant != "full":
                at_c = const.tile([128, MT, NT, 128], FP8, name="at_c")
                atc_f = at_c.rearrange("p a b c -> p (a b c)")
                half = MT * NT * 128 // 2
                nc.vector.memset(atc_f[:, :half], 0)
                nc.vector.memset(atc_f[:, half:], 0)
                g_c = const.tile([128, MT, C], BF16, name="g_c")
                nc.vector.memset(g_c.rearrange("p a b -> p (a b)"), 0)

            def body(_i=None):
                # ---- adjacency prefetch: 8 contiguous 1MB DMAs
                if variant == "full":
                    at = atp.tile([128, MT, NT, 128], FP8, name="at")
                else:
                    at = at_c
                atf = at.rearrange("p a b c -> p (a b c)")
                if variant != "mm":
                    engs = [nc.sync, nc.scalar, nc.gpsimd][:rings]
                    for ck in range(MT // mchunk):
                        w = mchunk * NT * 128
                        eng = engs[ck % rings]
                        eng.dma_start(out=atf[:, ck * w:(ck + 1) * w],
                                      in_=adjt[:, ck * w:(ck + 1) * w])

                # ---- preproc: G = [e*ft | e] straight from PSUM,
                # per 3-m-tile group
                g = gpool.tile([128, MT, C], BF16, name="g") \
                    if variant == "full" else g_c
                for k in range(NGRP if variant == "full" else 0):
                    m0 = k * GRP
                    nm = min(GRP, MT - m0)
                    psg = ps_g.tile([128, GRP, C], F32, name="psg", tag="psg")
                    for j in range(nm):
                        nc.tensor.matmul(
                            psg[:, j, :],
                            ft_sb[:, (m0 + j) * 128:(m0 + j + 1) * 128],
                            wc_sb, start=True, stop=True)
                    # e-cols: exp(PSUM) -> g[:, m0:m0+nm, 128:130] (bf16)
                    nc.scalar.activation(
                        _ap(g, m0 * C + 128, [[C, nm], [1, HEADS]]),
                        _ap(psg, 128, [[C, nm], [1, HEADS]]),
                        AF.Exp)
                    # ft-cols: psg * e (broadcast bf16 e-cols; their
                    # rounding cancels in the softmax ratio)
                    for h in range(HEADS):
                        nc.vector.tensor_mul(
                            _ap(g, m0 * C + h * 64, [[C, nm], [1, 64]]),
                            _ap(psg, h * 64, [[C, nm], [1, 64]]),
                            _ap(g, m0 * C + 128 + h, [[C, nm], [0, 64]]))

                # ---- main: single mt sweep, 8 accumulators in 3 banks
                ps_y = [ps_y_p.tile([128, GRP, C], F32, name=f"psy{b}",
                                    tag=f"psy{b}") for b in range(3)]
                for mt in range(MT):
                    for t in range(NT):
                        b, j = divmod(t, GRP)
                        nc.tensor.matmul(
                            ps_y[b][:, j, :], at[:, mt, t, :], g[:, mt, :],
                            start=(mt == 0 and j == 0),
                            stop=(mt == MT - 1),
                            skip_group_check=True)

                if variant != "full":
                    return
                # ---- epilogue: normalize (batched per bank), ELU, store
                obuf = ep.tile([128, NT, 128], F32, name="obuf")
                r2 = ep.tile([128, NT, HEADS], F32, name="r2")
                for b in range(3):
                    nt = min(GRP, NT - b * GRP)
                    nc.vector.reciprocal(
                        _ap(r2, b * GRP * HEADS, [[HEADS, nt], [1, HEADS]]),
                        _ap(ps_y[b], 128, [[C, nt], [1, HEADS]]))
                    nc.vector.tensor_mul(
                        _ap(obuf, b * GRP * 128,
                            [[128, nt], [64, HEADS], [1, 64]]),
                        _ap(ps_y[b], 0, [[C, nt], [64, HEADS], [1, 64]]),
                        _ap(r2, b * GRP * HEADS,
                            [[HEADS, nt], [1, HEADS], [0, 64]]))

                # ELU(x) = max(x, exp(min(x,0)) - 1)   (e^x >= 1+x)
                of = obuf.rearrange("p a b -> p (a b)")
                mn = ep.tile([128, NT * 128], BF16, name="mn")
                nc.vector.tensor_scalar_min(mn, of, 0.0)
                ex = ep.tile([128, NT * 128], F32, name="ex")
                nc.scalar.activation(ex, mn, AF.Exp)
                nc.vector.scalar_tensor_tensor(
                    of, ex, -1.0, of,
                    mybir.AluOpType.add, mybir.AluOpType.max)
                nc.sync.dma_start(out=out_pt, in_=obuf)

            if timing:
                assert reps % TIME_UNROLL == 0
                with tc.For_i(0, reps // TIME_UNROLL):
                    for _u in range(TIME_UNROLL):
                        body()
            else:
                for _rep in range(reps):
                    body()

    nc.compile()
    return nc


def make_in_maps(adj, features, W_attn, b_attn, W_lin, b_lin):
    adj = np.asarray(adj, dtype=np.float32)
    features = np.asarray(features, dtype=np.float32)
    W_attn = np.asarray(W_attn, dtype=np.float32)
    b_attn = np.asarray(b_attn, dtype=np.float32)
    W_lin = np.asarray(W_lin, dtype=np.float32)
    b_lin = np.asarray(b_lin, dtype=np.float32)

    BF = np.dtype(ml_dtypes.bfloat16)
    ftcat = np.concatenate([features.T,
                            np.ones((1, N), np.float32)], axis=0)
    ftcat = np.ascontiguousarray(ftcat).astype(BF)
    wcat = np.zeros((FT65, C), np.float32)
    wcat[:IN_DIM, 0:HEADS * OUT_DIM] = W_lin.T
    wcat[:IN_DIM, HEADS * OUT_DIM:] = W_attn[:, IN_DIM:].T
    wcat[IN_DIM, 0:HEADS * OUT_DIM] = b_lin
    wcat[IN_DIM, HEADS * OUT_DIM:] = b_attn
    wcat = wcat.astype(BF)

    A = adj.astype(NP_FP8)
    idx = np.arange(N)
    A[idx, idx] = (adj[idx, idx] + 1.0).astype(NP_FP8)

    in_maps = []
    for c in range(NCORES):
        slab = A[c * ROWS:(c + 1) * ROWS, :]       # [1024 n, 8192 m]
        # -> [128 (m in tile), MT, NT, 128 (n in tile)]
        adjt = slab.reshape(NT, 128, MT, 128).transpose(3, 2, 0, 1)
        adjt = np.ascontiguousarray(adjt).reshape(128, MT * NT * 128)
        in_maps.append({"adjt": adjt, "ftcat": ftcat, "wcat": wcat})
    return in_maps


_CACHED = {}


def _get_program(reps=1, timing=False, variant="full", mchunk=MCHUNK,
                 rings=2):
    key = (reps, timing, variant, mchunk, rings)
    if key not in _CACHED:
        _CACHED[key] = build_program(reps, timing=timing, variant=variant,
                                     mchunk=mchunk, rings=rings)
    return _CACHED[key]


def run_on_device(in_maps, reps=1, timing=False, variant="full",
                  mchunk=MCHUNK, rings=2, **kw):
    nc = _get_program(reps, timing=timing, variant=variant, mchunk=mchunk,
                      rings=rings)
    if timing:
        in_maps = [{k: v for k, v in m.items() if k != "adjt"}
                   for m in in_maps]
    res = run_bass_kernel_spmd(nc, in_maps, core_ids=list(range(NCORES)), **kw)
    return res


def kernel(adj, features, W_attn, b_attn, W_lin, b_lin):
    in_maps = make_in_maps(adj, features, W_attn, b_attn, W_lin, b_lin)
    res = run_on_device(in_maps, reps=1)
    return np.concatenate([res.results[c]["out"] for c in range(NCORES)],
                          axis=0)
